# revision 13
# baseline (speedup 1.0000x reference)
"""ConvDeepSet SPMD kernel for 8 Trainium2 NeuronCores.

Math (per batch b, all fp32 in reference):
    density = 1 where wt[:,0] finite else 0            [1,W,H]
    wt_aug  = concat([density, nan_to_num(wt)])        [CC=33,W,H]
    w0[w,x] = exp(-0.5*(lon_in[w]-lon_out[x])^2/ls^2)  [W,X]
    w1[h,y] = exp(-0.5*(lat_in[h]-lat_out[y])^2/ls^2)  [H,Y]
    ee[c,x,y] = sum_{w,h} wt_aug[c,w,h]*w0[w,x]*w1[h,y]
    out[0]   = ee[0];  out[c>=1] = ee[c] / clip(ee[0], 1e-6, 1e5)

Structural facts exploited (valid because the generated wt has no NaNs,
so density == 1 everywhere, and dens ~ 1e4 makes the clip a no-op):
  * dens[x,y] = s0[x]*s1[y] (s0 = w0.sum(0), s1 = w1.sum(0)): a rank-1
    outer product of small vectors -> computed EXACTLY on host.
  * 1/dens = r0[x]*r1[y] factors into the RBF weight matrices, so the
    device consumes pre-scaled w0*r0 / w1*r1 and the two chained
    matmuls produce normalized outputs directly; the reference's
    per-element division disappears.
  * w0/w1 are tiny (cheap host exp already needed for s0/s1), so they
    ship as inputs; the device runs ONLY the heavy per-channel
    contraction (3.7 GFLOP/core) plus PSUM drains and output DMA.

Sharding: data-parallel over batch B=8 -> one NeuronCore per batch.
Per-core, per channel c (32 channels, density excluded):
    stage1: T1[h, x] = wtr[:, c*H:(c+1)*H].T @ w0'    (contract W, 2 K)
    stage2: out[x, y] = T1[:, xs].T @ w1'             (contract H)

PSUM discipline: one pool of 4-bank [128, 2048] tiles (2 bufs = all 8
banks). A channel-pair's stage-1 lands in one tile (4 quadrants ->
ONE strided drain op); stage-2 packs 2 stripes x 2 channels per tile
(-> ONE drain op per 2 stripes). This minimizes DVE/ACT per-op
overhead on the PSUM->SBUF path (the roofline binder: PSUM reads are
1x on both engines; ~92k lane-elems/core must drain through them).
Drain ops alternate DVE/ACT ~48/52 for balance. Outputs stage in bf16
[X, C, Y] dram layout (2.9KB contiguous runs, 2 DMAs per 4-channel
group); host transposes, upcasts, prepends the exact density plane.
"""

import sys
from contextlib import ExitStack

import numpy as np

sys.path.insert(0, "/opt/trn_rl_repo")

import concourse.bass as bass  # noqa: E402,F401
import concourse.tile as tile  # noqa: E402
from concourse import bacc, mybir  # noqa: E402
from concourse.bass_utils import run_bass_kernel_spmd  # noqa: E402

B, C, W, H, X, Y = 8, 32, 256, 128, 720, 361
KT = W // 128       # stage-1 K tiles (2)
N1 = 360            # stage-1 moving split (720 = 2x360, <=512 per PSUM bank)
XOFF = [0, 128, 256, 384, 512, 616]   # stage-2 x stripes (4x128 + 2x104)
XLEN = [128, 128, 128, 128, 104, 104]
NXT = len(XOFF)
CG = 4              # output channels batched per DMA group

F32 = mybir.dt.float32
BF16 = mybir.dt.bfloat16

TRACE = False
LAST_RESULT = None

_cache = {}


def _build():
    nc = bacc.Bacc(
        "TRN2",
        target_bir_lowering=False,
        debug=False,
        enable_asserts=False,
        num_devices=B,
    )

    # wtr dram layout: [W, C*H] (w on rows); SBUF holds it as one tile
    # [128, KT*C*H] with the k-tile folded into the free dim so the whole
    # load is a few big contiguous-descriptor DMAs.
    wtr = nc.dram_tensor("wtr", [W, C * H], BF16, kind="ExternalInput").ap()
    # wcat: host-precomputed, normalization-scaled RBF weights, packed
    # [128, KT*X + Y]: w0' k-tiles then w1'.
    wcat = nc.dram_tensor("wcat", [128, KT * X + Y], BF16, kind="ExternalInput").ap()
    outc = nc.dram_tensor("outc", [X, C, Y], BF16, kind="ExternalOutput").ap()

    with tile.TileContext(nc) as tc, ExitStack() as ctx:
        wtr_pool = ctx.enter_context(tc.tile_pool(name="wtr", bufs=1))
        wc_pool = ctx.enter_context(tc.tile_pool(name="wc", bufs=1))
        t1sb_pool = ctx.enter_context(tc.tile_pool(name="t1sb", bufs=3))
        outsb_pool = ctx.enter_context(tc.tile_pool(name="outsb", bufs=2))
        ps_pool = ctx.enter_context(tc.tile_pool(name="ps", bufs=2, space="PSUM"))

        # ---- input loads. First the bits the first matmuls need, then the
        # rest; wtr chunks on the sync HWDGE queue, wcat on scalar's so the
        # head issues run in parallel.
        wtr_sb = wtr_pool.tile([128, KT * C * H], BF16, tag="wtr", name="wtr_sb")
        wdram = wtr.rearrange("(k p) f -> p k f", k=KT)
        wsb3 = wtr_sb[:].rearrange("p (k f) -> p k f", k=KT)
        wc_sb = wc_pool.tile([128, KT * X + Y], BF16, tag="wc", name="wc_sb")
        nc.sync.dma_start(
            wsb3[:, :, 0 : 2 * H], wdram[:, :, 0 : 2 * H]
        )
        nc.scalar.dma_start(wc_sb[:, 0:X], wcat[:, 0:X])
        nc.scalar.dma_start(wc_sb[:, X:], wcat[:, X:])
        for a, b in [(2, 8), (8, 20), (20, 32)]:
            nc.sync.dma_start(
                wsb3[:, :, a * H : b * H], wdram[:, :, a * H : b * H]
            )

        def wslice(k, c):
            return wtr_sb[:, k * C * H + c * H : k * C * H + (c + 1) * H]

        def w0slice(k, n):
            return wc_sb[:, k * X + n * N1 : k * X + (n + 1) * N1]

        w1_sb = wc_sb[:, KT * X : KT * X + Y]

        # Bresenham-style DVE/ACT alternation for the PSUM drains
        drain_ctr = [0]

        def drain(dst, src, frac_dve=0.48):
            drain_ctr[0] += 1
            if (drain_ctr[0] * 29) % 60 < 60 * frac_dve:
                nc.vector.tensor_copy(dst, src)
            else:
                nc.scalar.copy(dst, src)

        # ---- stage 1 for a channel pair: both channels' T1 in one 4-bank
        # psum tile (quadrants: c0n0, c0n1, c1n0, c1n1), k-outer so each
        # stationary is loaded once; ONE strided drain into the pair's
        # T1 sbuf tile [128, 2*X] (bf16).
        def stage1_pair(u):
            c0 = 2 * u
            t1ps = ps_pool.tile([128, 2048], F32, tag="ps", name=f"t1ps_u{u}")
            for k in range(KT):
                for idx in range(2):
                    for n in range(2):
                        nc.tensor.matmul(
                            t1ps[:, (2 * idx + n) * 512 : (2 * idx + n) * 512 + N1],
                            wslice(k, c0 + idx),
                            w0slice(k, n),
                            start=(k == 0),
                            stop=(k == KT - 1),
                        )
            t1sb = t1sb_pool.tile([128, 2 * X], BF16, tag="t1sb", name=f"t1sb_u{u}")
            src = t1ps[:].rearrange("p (q n) -> p q n", q=4)[:, :, 0:N1]
            dst = t1sb[:].rearrange("p (q n) -> p q n", q=4)
            drain(dst, src)
            return t1sb

        units = list(range(C // 2))
        stage_tiles = [None]

        def emit_stage2(u, t1sb):
            c0 = 2 * u
            g = c0 // CG
            ci0 = c0 % CG
            if ci0 == 0:
                stage_tiles[0] = outsb_pool.tile(
                    [128, NXT * CG * Y], BF16, tag="stage", name=f"stage_g{g}"
                )
            st = stage_tiles[0]
            st4 = st[:].rearrange("p (j c y) -> p j c y", j=NXT, c=CG)
            for jp in range(3):      # stripe pairs {0,1} {2,3} {4,5}
                j0 = 2 * jp
                xl0, xl1 = XLEN[j0], XLEN[j0 + 1]
                xl = max(xl0, xl1)
                eep = ps_pool.tile([128, 2048], F32, tag="ps", name=f"ee_u{u}_{jp}")
                for jd in range(2):
                    j = j0 + jd
                    for idx in range(2):
                        nc.tensor.matmul(
                            eep[0 : XLEN[j], (2 * jd + idx) * 512 : (2 * jd + idx) * 512 + Y],
                            t1sb[:, idx * X + XOFF[j] : idx * X + XOFF[j] + XLEN[j]],
                            w1_sb,
                            start=True,
                            stop=True,
                        )
                src = eep[0:xl, :].rearrange("p (q y) -> p q y", q=4)[:, :, 0:Y]
                dst = st4[0:xl, j0 : j0 + 2, ci0 : ci0 + 2, :]
                drain(dst, src)
            if ci0 + 2 == CG:
                # two DMAs per 4-channel group: x<512 (4 stripes) and the
                # 104/96 pair, alternating HWDGE queues
                eng = nc.sync if g % 2 == 0 else nc.scalar
                eng2 = nc.scalar if g % 2 == 0 else nc.sync
                d1 = outc[0:512, g * CG : (g + 1) * CG, :].rearrange(
                    "(j p) c y -> p j c y", j=4
                )
                eng.dma_start(d1, st4[:, 0:4])
                d2a = outc[512:616, g * CG : (g + 1) * CG, :]
                eng2.dma_start(d2a, st4[0:104, 4])
                d2b = outc[616:720, g * CG : (g + 1) * CG, :]
                eng2.dma_start(d2b, st4[0:104, 5])

        # software pipeline: emit stage1(u+1) before stage2(u) so the PE
        # works through the next pair while this one's psum drains.
        t1s = stage1_pair(units[0])
        for i, u in enumerate(units):
            t1s_next = stage1_pair(units[i + 1]) if i + 1 < len(units) else None
            emit_stage2(u, t1s)
            t1s = t1s_next

    nc.compile()
    return nc


def _reference_fallback(wt, x_in_lon, x_in_lat, x_out_lon, x_out_lat, init_ls):
    # Safety net for inputs with NaNs (never produced by the harness):
    # direct numpy evaluation of the reference formula.
    ls = float(np.asarray(init_ls).reshape(-1)[0])
    al = -0.5 / (ls * ls)
    density = (~np.isnan(wt[:, 0:1])).astype(np.float32)
    wta = np.concatenate([density, np.nan_to_num(wt, nan=0.0)], axis=1)
    w0 = np.exp(al * (x_in_lon[:, :, None] - x_out_lon[:, None, :]) ** 2)
    w1 = np.exp(al * (x_in_lat[:, :, None] - x_out_lat[:, None, :]) ** 2)
    t1 = np.einsum("bcwh,bwx->bcxh", wta, w0)
    ee = np.einsum("bcxh,bhy->bcxy", t1, w1)
    dens = ee[:, 0:1]
    return np.concatenate(
        [dens, ee[:, 1:] / np.clip(dens, 1e-6, 1e5)], axis=1
    ).astype(np.float32)


def kernel(wt, x_in_lon, x_in_lat, x_out_lon, x_out_lat, init_ls):
    global LAST_RESULT
    import ml_dtypes

    wt = np.asarray(wt, dtype=np.float32)
    x_in_lon = np.asarray(x_in_lon, dtype=np.float32)
    x_in_lat = np.asarray(x_in_lat, dtype=np.float32)
    x_out_lon = np.asarray(x_out_lon, dtype=np.float32)
    x_out_lat = np.asarray(x_out_lat, dtype=np.float32)
    ls = float(np.asarray(init_ls).reshape(-1)[0])
    alpha = -0.5 / (ls * ls)

    if np.isnan(wt).any():
        return _reference_fallback(
            wt, x_in_lon, x_in_lat, x_out_lon, x_out_lat, init_ls
        )

    # RBF weights + density plane + normalization on host (fp64).
    d0 = x_in_lon[:, :, None].astype(np.float64) - x_out_lon[:, None, :]
    w0 = np.exp(alpha * d0 * d0)                                  # [B, W, X]
    s0 = w0.sum(axis=1)                                           # [B, X]
    d1 = x_in_lat[:, :, None].astype(np.float64) - x_out_lat[:, None, :]
    w1 = np.exp(alpha * d1 * d1)                                  # [B, H, Y]
    s1 = w1.sum(axis=1)                                           # [B, Y]
    w0 = w0 / s0[:, None, :]
    w1 = w1 / s1[:, None, :]
    # pack [128, KT*X + Y] per batch (w-partition-major k-tiles, then w1)
    w0p = w0.reshape(B, KT, 128, X).transpose(0, 2, 1, 3).reshape(B, 128, KT * X)
    wcat = np.concatenate([w0p, w1], axis=2).astype(ml_dtypes.bfloat16)
    wcat = np.ascontiguousarray(wcat)

    # [B, C, W, H] -> [B, W, C*H] in bf16 (stage-1 stationary layout)
    wtr = np.ascontiguousarray(wt.transpose(0, 2, 1, 3)).reshape(B, W, C * H)
    wtr = wtr.astype(ml_dtypes.bfloat16)

    if "nc" not in _cache:
        _cache["nc"] = _build()
    nc = _cache["nc"]

    in_maps = [{"wtr": wtr[b], "wcat": wcat[b]} for b in range(B)]
    res = run_bass_kernel_spmd(nc, in_maps, list(range(B)), trace=TRACE)
    LAST_RESULT = res

    dens = (s0[:, :, None] * s1[:, None, :]).astype(np.float32)   # [B, X, Y]
    out = np.empty((B, C + 1, X, Y), dtype=np.float32)
    out[:, 0] = dens
    for b in range(B):
        oc = np.asarray(res.results[b]["outc"], dtype=np.float32)  # [X, C, Y]
        out[b, 1:] = oc.transpose(1, 0, 2)
    return out


# revision 14
# speedup vs baseline: 1.4276x; 1.4276x over previous
"""ConvDeepSet SPMD kernel for 8 Trainium2 NeuronCores.

Math (per batch b, all fp32 in reference):
    density = 1 where wt[:,0] finite else 0            [1,W,H]
    wt_aug  = concat([density, nan_to_num(wt)])        [CC=33,W,H]
    w0[w,x] = exp(-0.5*(lon_in[w]-lon_out[x])^2/ls^2)  [W,X]
    w1[h,y] = exp(-0.5*(lat_in[h]-lat_out[y])^2/ls^2)  [H,Y]
    ee[c,x,y] = sum_{w,h} wt_aug[c,w,h]*w0[w,x]*w1[h,y]
    out[0]   = ee[0];  out[c>=1] = ee[c] / clip(ee[0], 1e-6, 1e5)

Structural facts exploited (valid because the generated wt has no NaNs,
so density == 1 everywhere, and dens ~ 1e4 makes the clip a no-op):
  * dens[x,y] = s0[x]*s1[y] (s0 = w0.sum(0), s1 = w1.sum(0)): a rank-1
    outer product of small vectors -> computed EXACTLY on host.
  * 1/dens = r0[x]*r1[y] factors into the RBF weight matrices, so the
    device consumes pre-scaled w0*r0 / w1*r1 and the two chained
    matmuls produce normalized outputs directly; the reference's
    per-element division disappears.
  * w0/w1 are tiny (host already exps them for s0/s1), so they ship as
    inputs; the device runs ONLY the heavy per-channel contraction
    (3.7 GFLOP/core) plus PSUM drains and output DMA.

Sharding: data-parallel over batch B=8 -> one NeuronCore per batch.
Per-core, per channel c (32 channels, density excluded):
    stage1: T1[h, x] = wtr[:, c*H:(c+1)*H].T @ w0'    (contract W, 2 K)
    stage2: out[x, y] = T1[:, xs].T @ w1'             (contract H)

The binder is the PSUM->SBUF drain path (PSUM reads are 1x on DVE and
ACT; ~92k lane-elems/core must leave PSUM through those two engines,
~60us each when balanced). Everything is arranged around keeping both
drain engines saturated: a single rotating pool of four 2-bank PSUM
tiles (stage-1: one channel's T1 halves; stage-2: one stripe of a
channel pair), drains alternating strictly DVE/ACT, ACT freed of all
DMA issue work (inputs+small outputs on the sync HWDGE queue, big
output blocks on the idle GPSIMD/SWDGE path), and output DMAs issued
progressively per stripe-pair so the final group drains a short tail.
Outputs stage in bf16 [X, C, Y] dram layout (2.9KB contiguous runs);
host transposes, upcasts, prepends the exact density plane.
"""

import sys
from contextlib import ExitStack

import numpy as np

sys.path.insert(0, "/opt/trn_rl_repo")

import concourse.bass as bass  # noqa: E402,F401
import concourse.tile as tile  # noqa: E402
from concourse import bacc, mybir  # noqa: E402
from concourse.bass_utils import run_bass_kernel_spmd  # noqa: E402

B, C, W, H, X, Y = 8, 32, 256, 128, 720, 361
KT = W // 128       # stage-1 K tiles (2)
N1 = 360            # stage-1 moving split (720 = 2x360, <=512 per PSUM bank)
XOFF = [0, 128, 256, 384, 512, 616]   # stage-2 x stripes (4x128 + 2x104)
XLEN = [128, 128, 128, 128, 104, 104]
NXT = len(XOFF)
CG = 4              # output channels batched per DMA group

F32 = mybir.dt.float32
BF16 = mybir.dt.bfloat16

TRACE = False
LAST_RESULT = None

_cache = {}


def _build():
    nc = bacc.Bacc(
        "TRN2",
        target_bir_lowering=False,
        debug=False,
        enable_asserts=False,
        num_devices=B,
    )

    wtr = nc.dram_tensor("wtr", [W, C * H], BF16, kind="ExternalInput").ap()
    # wcat: host-precomputed, normalization-scaled RBF weights, packed
    # [128, KT*X + Y]: w0' k-tiles then w1'.
    wcat = nc.dram_tensor("wcat", [128, KT * X + Y], BF16, kind="ExternalInput").ap()
    outc = nc.dram_tensor("outc", [X, C, Y], BF16, kind="ExternalOutput").ap()

    with tile.TileContext(nc) as tc, ExitStack() as ctx:
        wtr_pool = ctx.enter_context(tc.tile_pool(name="wtr", bufs=1))
        wc_pool = ctx.enter_context(tc.tile_pool(name="wc", bufs=1))
        t1sb_pool = ctx.enter_context(tc.tile_pool(name="t1sb", bufs=6))
        outsb_pool = ctx.enter_context(tc.tile_pool(name="outsb", bufs=2))
        ps_pool = ctx.enter_context(tc.tile_pool(name="ps", bufs=4, space="PSUM"))

        # ---- input loads, all on the sync HWDGE queue, critical bits first
        # (the first matmuls need w0' k0 + channels 0-1; issue cost is
        # ~0.7us per dma_start so the order matters).
        wtr_sb = wtr_pool.tile([128, KT * C * H], BF16, tag="wtr", name="wtr_sb")
        wdram = wtr.rearrange("(k p) f -> p k f", k=KT)
        wsb3 = wtr_sb[:].rearrange("p (k f) -> p k f", k=KT)
        wc_sb = wc_pool.tile([128, KT * X + Y], BF16, tag="wc", name="wc_sb")
        nc.sync.dma_start(wc_sb[:, 0:X], wcat[:, 0:X])
        nc.sync.dma_start(wsb3[:, :, 0 : 2 * H], wdram[:, :, 0 : 2 * H])
        nc.sync.dma_start(wc_sb[:, X:], wcat[:, X:])
        for a, b in [(2, 8), (8, 20), (20, 32)]:
            nc.sync.dma_start(
                wsb3[:, :, a * H : b * H], wdram[:, :, a * H : b * H]
            )

        def wslice(k, c):
            return wtr_sb[:, k * C * H + c * H : k * C * H + (c + 1) * H]

        def w0slice(k, n):
            return wc_sb[:, k * X + n * N1 : k * X + (n + 1) * N1]

        w1_sb = wc_sb[:, KT * X : KT * X + Y]

        # strict DVE/ACT alternation for the PSUM drains
        drain_ctr = [0]

        def drain(dst, src):
            drain_ctr[0] += 1
            if drain_ctr[0] % 2 == 0:
                nc.vector.tensor_copy(dst, src)
            else:
                nc.scalar.copy(dst, src)

        # ---- stage 1 for one channel: T1 halves in one 2-bank psum tile
        # (k-outer: each wtr stationary loaded once), ONE strided drain
        # into the channel's T1 sbuf tile [128, X] (bf16).
        def stage1(c):
            t1ps = ps_pool.tile([128, 1024], F32, tag="ps", name=f"t1ps_c{c}")
            for k in range(KT):
                for n in range(2):
                    nc.tensor.matmul(
                        t1ps[:, n * 512 : n * 512 + N1],
                        wslice(k, c),
                        w0slice(k, n),
                        start=(k == 0),
                        stop=(k == KT - 1),
                    )
            t1sb = t1sb_pool.tile([128, X], BF16, tag="t1sb", name=f"t1sb_c{c}")
            src = t1ps[:].rearrange("p (q n) -> p q n", q=2)[:, :, 0:N1]
            dst = t1sb[:].rearrange("p (q n) -> p q n", q=2)
            drain(dst, src)
            return t1sb

        units = list(range(C // 2))
        stage_tiles = [None]

        def emit_stage2(u, t1sbs):
            c0 = 2 * u
            g = c0 // CG
            ci0 = c0 % CG
            if ci0 == 0:
                stage_tiles[0] = outsb_pool.tile(
                    [128, NXT * CG * Y], BF16, tag="stage", name=f"stage_g{g}"
                )
            st = stage_tiles[0]
            st4 = st[:].rearrange("p (j c y) -> p j c y", j=NXT, c=CG)
            for j in range(NXT):
                xo, xl = XOFF[j], XLEN[j]
                eep = ps_pool.tile([128, 1024], F32, tag="ps", name=f"ee_u{u}_{j}")
                for idx in range(2):
                    nc.tensor.matmul(
                        eep[0:xl, idx * 512 : idx * 512 + Y],
                        t1sbs[idx][:, xo : xo + xl],
                        w1_sb,
                        start=True,
                        stop=True,
                    )
                src = eep[0:xl, :].rearrange("p (q y) -> p q y", q=2)[:, :, 0:Y]
                dst = st4[0:xl, j, ci0 : ci0 + 2, :]
                drain(dst, src)
                if ci0 + 2 == CG and j % 2 == 1:
                    # progressive output DMAs per (group, stripe pair):
                    # the two x<512 pairs ride the idle SWDGE path, the
                    # 104-col pair as two small HWDGE transfers.
                    if j < 4:
                        dd = outc[
                            XOFF[j - 1] : XOFF[j] + XLEN[j], g * CG : (g + 1) * CG, :
                        ].rearrange("(s p) c y -> p s c y", s=2)
                        sb = st4[:, j - 1 : j + 1]
                        nc.gpsimd.dma_start(dd, sb)
                    else:
                        nc.sync.dma_start(
                            outc[512:616, g * CG : (g + 1) * CG, :], st4[0:104, 4]
                        )
                        nc.sync.dma_start(
                            outc[616:720, g * CG : (g + 1) * CG, :], st4[0:104, 5]
                        )

        # software pipeline: emit stage1(u+1) before stage2(u) so the PE
        # works through the next pair while this one's psum drains.
        t1s = [stage1(2 * units[0]), stage1(2 * units[0] + 1)]
        for i, u in enumerate(units):
            t1s_next = (
                [stage1(2 * units[i + 1]), stage1(2 * units[i + 1] + 1)]
                if i + 1 < len(units)
                else None
            )
            emit_stage2(u, t1s)
            t1s = t1s_next

    nc.compile()
    return nc


def _reference_fallback(wt, x_in_lon, x_in_lat, x_out_lon, x_out_lat, init_ls):
    # Safety net for inputs with NaNs (never produced by the harness):
    # direct numpy evaluation of the reference formula.
    ls = float(np.asarray(init_ls).reshape(-1)[0])
    al = -0.5 / (ls * ls)
    density = (~np.isnan(wt[:, 0:1])).astype(np.float32)
    wta = np.concatenate([density, np.nan_to_num(wt, nan=0.0)], axis=1)
    w0 = np.exp(al * (x_in_lon[:, :, None] - x_out_lon[:, None, :]) ** 2)
    w1 = np.exp(al * (x_in_lat[:, :, None] - x_out_lat[:, None, :]) ** 2)
    t1 = np.einsum("bcwh,bwx->bcxh", wta, w0)
    ee = np.einsum("bcxh,bhy->bcxy", t1, w1)
    dens = ee[:, 0:1]
    return np.concatenate(
        [dens, ee[:, 1:] / np.clip(dens, 1e-6, 1e5)], axis=1
    ).astype(np.float32)


def kernel(wt, x_in_lon, x_in_lat, x_out_lon, x_out_lat, init_ls):
    global LAST_RESULT
    import ml_dtypes

    wt = np.asarray(wt, dtype=np.float32)
    x_in_lon = np.asarray(x_in_lon, dtype=np.float32)
    x_in_lat = np.asarray(x_in_lat, dtype=np.float32)
    x_out_lon = np.asarray(x_out_lon, dtype=np.float32)
    x_out_lat = np.asarray(x_out_lat, dtype=np.float32)
    ls = float(np.asarray(init_ls).reshape(-1)[0])
    alpha = -0.5 / (ls * ls)

    if np.isnan(wt).any():
        return _reference_fallback(
            wt, x_in_lon, x_in_lat, x_out_lon, x_out_lat, init_ls
        )

    # RBF weights + density plane + normalization on host (fp64).
    d0 = x_in_lon[:, :, None].astype(np.float64) - x_out_lon[:, None, :]
    w0 = np.exp(alpha * d0 * d0)                                  # [B, W, X]
    s0 = w0.sum(axis=1)                                           # [B, X]
    d1 = x_in_lat[:, :, None].astype(np.float64) - x_out_lat[:, None, :]
    w1 = np.exp(alpha * d1 * d1)                                  # [B, H, Y]
    s1 = w1.sum(axis=1)                                           # [B, Y]
    w0 = w0 / s0[:, None, :]
    w1 = w1 / s1[:, None, :]
    # pack [128, KT*X + Y] per batch (w-partition-major k-tiles, then w1)
    w0p = w0.reshape(B, KT, 128, X).transpose(0, 2, 1, 3).reshape(B, 128, KT * X)
    wcat = np.concatenate([w0p, w1], axis=2).astype(ml_dtypes.bfloat16)
    wcat = np.ascontiguousarray(wcat)

    # [B, C, W, H] -> [B, W, C*H] in bf16 (stage-1 stationary layout)
    wtr = np.ascontiguousarray(wt.transpose(0, 2, 1, 3)).reshape(B, W, C * H)
    wtr = wtr.astype(ml_dtypes.bfloat16)

    if "nc" not in _cache:
        _cache["nc"] = _build()
    nc = _cache["nc"]

    in_maps = [{"wtr": wtr[b], "wcat": wcat[b]} for b in range(B)]
    res = run_bass_kernel_spmd(nc, in_maps, list(range(B)), trace=TRACE)
    LAST_RESULT = res

    dens = (s0[:, :, None] * s1[:, None, :]).astype(np.float32)   # [B, X, Y]
    out = np.empty((B, C + 1, X, Y), dtype=np.float32)
    out[:, 0] = dens
    for b in range(B):
        oc = np.asarray(res.results[b]["outc"], dtype=np.float32)  # [X, C, Y]
        out[b, 1:] = oc.transpose(1, 0, 2)
    return out
